# revision 56
# baseline (speedup 1.0000x reference)
"""Trainium2 Bass kernel for nn_AttentionBlock (B=16, C=512, H=W=32, 4 heads).

Data-parallel over batch across 8 NeuronCores (2 images/core), no
collectives. All GEMMs run in fp8 (e4m3) with perf_mode=DoubleRow: both
operands are laid out as [K<=128 partitions, 2, free] so each matmul
contracts 2*K rows at 0.5 PE-cycles per output row. Per image (x[b] is
(C, N) feature-major, N = H*W = 1024 tokens):

  q,k   = wqk^T @ x          feature-major, stored bf16 d-major per head
                             (bf16 storage + bf16 score matmuls cost PE time
                             that hides under the ACT/DVE wall but halve the
                             attention-logit quantization error)
  v     = x^T @ wv           token-major [j, C]
  sT    = kT^T @ qT          (j, i) layout, bf16, one matmul per j-tile
  e     = exp(sT*scale - 5)  ACT, fp8 out; constant shift keeps e in [0, ~15]
                             (softmax is shift-invariant; D uses the same e)
  D     = ones^T @ e         column sums via fp8 DR matmul, fp32 PSUM
  res   = (v^T @ e) / D      PV via DR matmul; DVE reciprocal + multiply
  y     = wout^T @ res + b_eff + x

With fp8 matmuls the kernel is bound by the scalar engine's exp stream
(~66us of ACT busy for 2 * 4 * 1024^2 exps/core), so the schedule is built
around keeping ACT saturated end to end:
  - exp consumes score PSUM in [128, 2, 512] bank pairs (2-deep ring);
  - the two heads of a pair interleave their score/exp streams, with 4 e8
    buffers so PV/normalize never blocks the next head's exps;
  - x ships pre-cast to fp8 (host) so the first scores need only 2 small
    DMAs + 2 matmul-pairs + 4 evacuations; the critical wqk chunks are a
    contiguous early DMA slice (host chunk reorder), and 2 of the first
    evacuations run on the then-idle ACT as identity+bias;
  - PSUM->SBUF evacuations order: the 6 chunk-evacs heads 2/3 need come
    before the v copies, so DVE never starves ACT mid-stream;
  - out-projection overlaps the next token-block's attention; its PSUM
    shares the D/PV rings (alternating) and the final stores split across
    the SP and ACT DMA queues at the tail;
  - 4 score-pairs of the second batch compute their exp on DVE+gpsimd
    instead (Schraudolph int32-affine + bitcast, fp8 convert on gpsimd),
    using the projection PSUM ring which is idle during that window --
    this drains ~4us off the ACT stream without touching its score ring;
  - at the very tail, two of the four output tiles evacuate via ACT
    identity(+bias) + gpsimd residual-add, halving the DVE-serial epilogue.

Bias handling: q/k biases are added at PSUM->SBUF evacuation (per-partition
scalars); the v bias commutes through the softmax average and folds into
b_eff = b_out + b_v @ w_out on the host. The fp32 x is used only for the
residual add (full precision); the fp8 x copy only feeds the projections,
which quantize to fp8 anyway.
"""

import numpy as np
import ml_dtypes

import concourse.mybir as mybir
import concourse.tile as tile
from concourse import bacc
from concourse.bass_utils import run_bass_kernel_spmd

dt = mybir.dt
F8NP = ml_dtypes.float8_e4m3
DR = mybir.MatmulPerfMode.DoubleRow

N_CORES = 8
B = 16
C = 512
HEADS = 4
DK = C // HEADS          # 128
N = 1024                 # H*W tokens
SCALE = float(DK) ** -0.5
SHIFT = -5.0             # exp(s*scale + SHIFT) <= ~15 fits e4m3 (max 240)
# Schraudolph fast-exp constants (int32 affine + bitcast) for score tiles
# offloaded to DVE when ACT is the bottleneck and DVE has idle
SCH_A = 12102203.161561485 * SCALE       # (2^23/ln2) * scale
SCH_B = 127 * (1 << 23) - 366393 + 12102203.161561485 * SHIFT
BPC = B // N_CORES       # batches per core = 2
CT = C // 128            # 4 contraction tiles over channels
CTP = CT // 2            # 2 DoubleRow contraction pairs
NB = N // 512            # 2 moving-dim blocks of 512 tokens
JT = N // 128            # 8 key-token tiles
JP = JT // 2             # 4 DoubleRow j-tile pairs

LAST_RESULTS = None  # BassKernelResults of the most recent run (for test.py)


def build_program():
    nc = bacc.Bacc("TRN2", target_bir_lowering=False, debug=False,
                   num_devices=N_CORES)

    x = nc.dram_tensor("x", [BPC, C, N], dt.float32, kind="ExternalInput").ap()
    x8 = nc.dram_tensor("x8", [BPC, C, N], dt.float8e4, kind="ExternalInput").ap()
    # contraction index c = ctp*256 + ko*128 + p for all three weights
    wqk = nc.dram_tensor("wqk", [128, CTP, 2, 8, 128], dt.float8e4,
                         kind="ExternalInput").ap()
    wv = nc.dram_tensor("wv", [128, CTP, 2, C], dt.float8e4,
                        kind="ExternalInput").ap()
    wout = nc.dram_tensor("wout", [128, CTP, 2, CT, 128], dt.float8e4,
                          kind="ExternalInput").ap()
    ones = nc.dram_tensor("ones", [128, 2, 128], dt.float8e4,
                          kind="ExternalInput").ap()
    bqk = nc.dram_tensor("bqk", [128, 8], dt.float32, kind="ExternalInput").ap()
    beff = nc.dram_tensor("beff", [128, CT], dt.float32,
                          kind="ExternalInput").ap()
    y = nc.dram_tensor("y", [BPC, C, N], dt.float32, kind="ExternalOutput").ap()

    with tile.TileContext(nc) as tc:
        with (
            tc.tile_pool(name="weights", bufs=1) as wpool,
            tc.tile_pool(name="xin", bufs=2) as xpool,
            tc.tile_pool(name="x8", bufs=2) as x8pool,
            tc.tile_pool(name="qk", bufs=2) as qkpool,
            tc.tile_pool(name="vbuf", bufs=2) as vpool,
            tc.tile_pool(name="ebuf", bufs=4) as epool,
            tc.tile_pool(name="dbuf", bufs=2) as dpool,
            tc.tile_pool(name="ibuf", bufs=2) as ipool,
            tc.tile_pool(name="res", bufs=2) as rpool,
            tc.tile_pool(name="yout", bufs=4) as ypool,
            tc.tile_pool(name="ps_s", bufs=2, space="PSUM") as ps_s,
            tc.tile_pool(name="ps_p", bufs=2, space="PSUM") as ps_p,
            tc.tile_pool(name="ps_d", bufs=1, space="PSUM") as ps_d,
            tc.tile_pool(name="ps_r", bufs=1, space="PSUM") as ps_r,
        ):
            # ---- loads: x8 (fp8, host-cast) feeds the projections and is
            # on the critical path; fp32 x is residual-only and loads late.
            xT_sbs, x8_sbs = [], []
            for b in range(BPC):
                xT_sb = xpool.tile([128, CT, NB, 512], dt.float32)
                x8_sb = x8pool.tile([128, CT, NB, 512], dt.float8e4)
                xT_sbs.append(xT_sb)
                x8_sbs.append(x8_sb)

            shift_sb = wpool.tile([128, 1], dt.float32)
            nc.any.memset(shift_sb, SHIFT)
            # pre-warm the ACT exp table so the first real exp doesn't pay
            # the ~1.3us table load on the critical path
            warm_sb = wpool.tile([128, 1], dt.float32)
            nc.scalar.activation(warm_sb, shift_sb,
                                 mybir.ActivationFunctionType.Exp, scale=0.0)
            x8r = [x8[b].rearrange("(ct p) (nb n) -> p ct nb n", p=128, n=512)
                   for b in range(BPC)]
            # tiny bias DMA rides the idle ACT HWDGE queue at t=0
            bqk_sb = wpool.tile([128, 8], dt.float32)
            nc.scalar.dma_start(out=bqk_sb, in_=bqk)
            nc.sync.dma_start(out=x8_sbs[0][:, :, 0, :], in_=x8r[0][:, :, 0, :])
            # wqk/bqk chunk axis is host-reordered to (0,1,4,5,2,3,6,7) so
            # the critical chunks are one contiguous early DMA slice
            wqk_sb = wpool.tile([128, CTP, 2, 8, 128], dt.float8e4)
            nc.sync.dma_start(out=wqk_sb[:, :, :, 0:4, :],
                              in_=wqk[:, :, :, 0:4, :])
            nc.sync.dma_start(out=x8_sbs[0][:, :, 1, :], in_=x8r[0][:, :, 1, :])
            nc.sync.dma_start(out=wqk_sb[:, :, :, 4:8, :],
                              in_=wqk[:, :, :, 4:8, :])
            wv_sb = wpool.tile([128, CTP, 2, C], dt.float8e4)
            nc.sync.dma_start(out=wv_sb, in_=wv)
            wout_sb = wpool.tile([128, CTP, 2, CT, 128], dt.float8e4)
            nc.sync.dma_start(out=wout_sb, in_=wout)
            ones_sb = wpool.tile([128, 2, 128], dt.float8e4)
            nc.sync.dma_start(out=ones_sb, in_=ones)
            beff_sb = wpool.tile([128, CT], dt.float32)
            nc.sync.dma_start(out=beff_sb, in_=beff)
            nc.sync.dma_start(out=x8_sbs[1], in_=x8r[1])
            xr = [x[b].rearrange("(ct p) (nb n) -> p ct nb n", p=128, n=512)
                  for b in range(BPC)]
            nc.sync.dma_start(out=xT_sbs[0], in_=xr[0])
            nc.sync.dma_start(out=xT_sbs[1], in_=xr[1])

            for b in range(BPC):
                xT_sb, x8_sb = xT_sbs[b], x8_sbs[b]
                # ---- q/k + v projections, interleaved so that heads
                # 0/1's score tiles unlock as early as possible ----
                # qk8[(p<64 ? head 2hp : head 2hp+1), qk, hp, t, nb, n];
                # head-dim index d = t*64 + (p mod 64)
                qk8 = qkpool.tile([128, 2, HEADS, NB, 512], dt.bfloat16)
                v8 = vpool.tile([128, JP, 2, C], dt.float8e4)

                # chunk order (0,1,4,5,2,3,6,7) = (q,h0),(q,h1),(k,h0),
                # (k,h1),(q,h2),(q,h3),(k,h2),(k,h3): heads 0/1 first
                CHUNK_POS = {0: 0, 1: 1, 4: 2, 5: 3, 2: 4, 3: 5, 6: 6, 7: 7}

                def qk_chunk(n_ch, nb, on_act=False):
                    qk, h = n_ch >> 2, n_ch & 3
                    pos = CHUNK_POS[n_ch]
                    ps = ps_p.tile([128, 512], dt.float32, tag="psp")
                    for ctp in range(CTP):
                        nc.tensor.matmul(
                            ps,
                            wqk_sb[:, ctp, :, pos, :],
                            x8_sb[:, 2 * ctp:2 * ctp + 2, nb, :],
                            start=(ctp == 0), stop=(ctp == CTP - 1),
                            perf_mode=DR)
                    if on_act:
                        # ACT is idle before the first exp; identity+bias
                        # shares the exp table (no table reload)
                        nc.scalar.activation(
                            qk8[:, qk, h, nb, :], ps,
                            mybir.ActivationFunctionType.Identity,
                            bias=bqk_sb[:, pos:pos + 1])
                    else:
                        nc.vector.tensor_scalar_add(
                            qk8[:, qk, h, nb, :], ps,
                            bqk_sb[:, pos:pos + 1])

                def v_tile(jt):
                    ps = ps_p.tile([128, 512], dt.float32, tag="psp")
                    nbj, off = divmod(jt * 128, 512)
                    for ctp in range(CTP):
                        nc.tensor.matmul(
                            ps,
                            x8_sb[:, 2 * ctp:2 * ctp + 2, nbj, off:off + 128],
                            wv_sb[:, ctp, :, :],
                            start=(ctp == 0), stop=(ctp == CTP - 1),
                            perf_mode=DR)
                    nc.vector.tensor_copy(v8[:, jt // 2, jt % 2, :], ps)

                # ---- attention emission helpers ----
                yr = y[b].rearrange("(cot p) (nb n) -> p cot nb n",
                                    p=128, n=512)
                res8 = rpool.tile([128, CT, NB, 512], dt.float8e4)

                def scores_pair(h, ib, jps, e8):
                    ps = ps_s.tile([128, 2, 512], dt.float32, tag="ps")
                    for half in range(2):
                        jt = 2 * jps + half
                        nbj, off = divmod(jt * 128, 512)
                        nc.tensor.matmul(
                            ps[:, half, :],
                            qk8[:, 1, h, nbj, off:off + 128],
                            qk8[:, 0, h, ib, :],
                            start=True, stop=True)
                    nc.scalar.activation(
                        e8[:, 2 * jps:2 * jps + 2, :], ps,
                        mybir.ActivationFunctionType.Exp,
                        scale=SCALE, bias=shift_sb)

                def pv_norm(h, ib, e8):
                    psd = ps_d.tile([128, 512], dt.float32, tag="psd")
                    psr = ps_r.tile([128, 512], dt.float32, tag="psr")
                    for jp2 in range(JP):
                        epair = e8[:, 2 * jp2:2 * jp2 + 2, :]
                        nc.tensor.matmul(
                            psd, ones_sb, epair,
                            start=(jp2 == 0), stop=(jp2 == JP - 1),
                            perf_mode=DR)
                        nc.tensor.matmul(
                            psr, v8[:, jp2, :, h * DK:(h + 1) * DK], epair,
                            start=(jp2 == 0), stop=(jp2 == JP - 1),
                            perf_mode=DR)
                    d_sb = dpool.tile([128, 512], dt.float32)
                    nc.vector.reciprocal(d_sb, psd)
                    nc.vector.tensor_mul(res8[:, h, ib, :], psr, d_sb)

                def scores_pair_schr(h, ib, jps, e8):
                    # DVE-side fast exp: the score matmuls land in the (idle
                    # during this batch's attention) psp ring, so ACT's score
                    # ring is untouched by the offload
                    for half in range(2):
                        jt = 2 * jps + half
                        nbj, off = divmod(jt * 128, 512)
                        ps = ps_p.tile([128, 512], dt.float32, tag="psp")
                        nc.tensor.matmul(
                            ps,
                            qk8[:, 1, h, nbj, off:off + 128],
                            qk8[:, 0, h, ib, :],
                            start=True, stop=True)
                        i_sb = ipool.tile([128, 512], dt.int32)
                        nc.vector.tensor_scalar(
                            i_sb, ps, SCH_A, SCH_B,
                            op0=mybir.AluOpType.mult, op1=mybir.AluOpType.add)
                        # int32->fp8 convert is SBUF->SBUF: run it on the
                        # otherwise-idle gpsimd engine
                        nc.gpsimd.tensor_copy(e8[:, jt, :],
                                              i_sb.bitcast(dt.float32))

                def attn_head_pair(ha, hb, ib, schr=()):
                    # interleave the two heads' score/exp streams so the ACT
                    # pipeline never waits on one head's input chain
                    e8a = epool.tile([128, JT, 512], dt.float8e4, tag="e8")
                    e8b = epool.tile([128, JT, 512], dt.float8e4, tag="e8")
                    for jps in range(JP):
                        if (0, jps) in schr:
                            scores_pair_schr(ha, ib, jps, e8a)
                        else:
                            scores_pair(ha, ib, jps, e8a)
                        if (1, jps) in schr:
                            scores_pair_schr(hb, ib, jps, e8b)
                        else:
                            scores_pair(hb, ib, jps, e8b)
                    pv_norm(ha, ib, e8a)
                    pv_norm(hb, ib, e8b)

                def out_proj(ib, ctp_order=(0, 1), split_dma=False):
                    # ctp_order: put the earlier-finishing head pair first so
                    # the accumulation can start before the last mul lands
                    for cot in range(CT):
                        pool_o = ps_d if cot % 2 == 0 else ps_r
                        ps = pool_o.tile([128, 512], dt.float32,
                                         tag="psd" if cot % 2 == 0 else "psr")
                        for k, ctp in enumerate(ctp_order):
                            nc.tensor.matmul(
                                ps,
                                wout_sb[:, ctp, :, cot, :],
                                res8[:, 2 * ctp:2 * ctp + 2, ib, :],
                                start=(k == 0), stop=(k == CTP - 1),
                                perf_mode=DR)
                        y_sb = ypool.tile([128, 512], dt.float32)
                        if split_dma and cot % 2:
                            # tail: ACT and Pool are idle; evacuate via ACT
                            # identity(+bias) and add the residual on Pool so
                            # the DVE-serial stt chain halves
                            y_tmp = ypool.tile([128, 512], dt.float32,
                                               tag="ytmp")
                            nc.scalar.activation(
                                y_tmp, ps,
                                mybir.ActivationFunctionType.Identity,
                                bias=beff_sb[:, cot:cot + 1])
                            nc.gpsimd.tensor_add(y_sb, y_tmp,
                                                 xT_sb[:, cot, ib, :])
                            nc.scalar.dma_start(out=yr[:, cot, ib, :],
                                                in_=y_sb)
                        else:
                            nc.vector.scalar_tensor_tensor(
                                y_sb, ps, beff_sb[:, cot:cot + 1],
                                xT_sb[:, cot, ib, :],
                                op0=mybir.AluOpType.add,
                                op1=mybir.AluOpType.add)
                            nc.sync.dma_start(out=yr[:, cot, ib, :], in_=y_sb)

                # ---- emission order: critical projections, heads 0/1,
                # remaining projections, heads 2/3, then ib=1 ----
                for n_ch, nb in ((0, 0), (1, 0), (4, 0), (5, 0), (4, 1),
                                 (5, 1)):
                    qk_chunk(n_ch, nb, on_act=(b == 0 and nb == 0
                                               and n_ch in (4, 5)))
                for n_ch, nb in ((2, 0), (3, 0), (6, 0), (7, 0), (6, 1),
                                 (7, 1)):
                    qk_chunk(n_ch, nb)
                for jt in range(JT):
                    v_tile(jt)
                for n_ch, nb in ((0, 1), (1, 1), (2, 1), (3, 1)):
                    qk_chunk(n_ch, nb)
                SCHR0 = ((0, 1),) if b == 1 else ()
                SCHR1 = ((1, 2), (0, 2), (1, 1)) if b == 1 else ()
                attn_head_pair(0, 1, 0, schr=SCHR0)
                attn_head_pair(2, 3, 0, schr=SCHR0)
                out_proj(0)
                attn_head_pair(2, 3, 1, schr=SCHR1)
                attn_head_pair(0, 1, 1)
                out_proj(1, ctp_order=(1, 0), split_dma=(b == BPC - 1))
    nc.finalize()
    return nc


_CACHED_NC = None


def _get_program():
    global _CACHED_NC
    if _CACHED_NC is None:
        _CACHED_NC = build_program()
    return _CACHED_NC


def _pack_weights(w_proj, b_proj, w_out, b_out):
    w4 = w_proj.reshape(C, HEADS, 3, DK)
    # wqk8[c -> (ctp, ko, p), n_ch=(qk, h), d]: q/k chunks are d-major so
    # scores run as plain bf16 matmuls with d on partitions
    arr = np.empty((C, 2, HEADS, 128), np.float32)
    for qk in range(2):
        for h in range(HEADS):
            arr[:, qk, h, :] = w4[:, h, qk, :]
    wqk8 = np.ascontiguousarray(
        arr.reshape(CTP, 2, 128, 8, 128).transpose(2, 0, 1, 3, 4))
    CHUNK_ORDER = (0, 1, 4, 5, 2, 3, 6, 7)
    wqk8 = np.ascontiguousarray(wqk8[:, :, :, CHUNK_ORDER, :]).astype(F8NP)

    b4 = b_proj.reshape(HEADS, 3, DK)
    bqk = np.empty((128, 8), np.float32)
    for n_ch in range(8):
        qk, h = n_ch >> 2, n_ch & 3
        bqk[:, n_ch] = b4[h, qk, :]

    wv = w4[:, :, 2, :].reshape(C, C)
    wv8 = np.ascontiguousarray(
        wv.reshape(CTP, 2, 128, C).transpose(2, 0, 1, 3)).astype(F8NP)

    wout8 = np.ascontiguousarray(
        w_out.reshape(CTP, 2, 128, CT, 128).transpose(2, 0, 1, 3, 4)).astype(F8NP)

    # v-bias commutes through the softmax average: b_eff = b_out + b_v @ w_out
    b_eff = b_out + b4[:, 2, :].reshape(C) @ w_out
    beff = np.ascontiguousarray(b_eff.reshape(CT, 128).T)

    bqk = bqk[:, list(CHUNK_ORDER)]
    return {
        "wqk": wqk8, "wv": wv8, "wout": wout8,
        "ones": np.ones((128, 2, 128), np.float32).astype(F8NP),
        "bqk": np.ascontiguousarray(bqk), "beff": beff,
    }


def kernel(x, w_proj, b_proj, w_out, b_out):
    global LAST_RESULTS
    x = np.ascontiguousarray(np.asarray(x, dtype=np.float32)).reshape(B, C, N)
    w_proj = np.asarray(w_proj, dtype=np.float32)
    b_proj = np.asarray(b_proj, dtype=np.float32)
    w_out = np.asarray(w_out, dtype=np.float32)
    b_out = np.asarray(b_out, dtype=np.float32)

    weights = _pack_weights(w_proj, b_proj, w_out, b_out)

    x8 = x.astype(F8NP)
    nc = _get_program()
    in_maps = []
    for c in range(N_CORES):
        in_maps.append({"x": x[c * BPC:(c + 1) * BPC],
                        "x8": x8[c * BPC:(c + 1) * BPC], **weights})
    res = run_bass_kernel_spmd(nc, in_maps, list(range(N_CORES)))
    LAST_RESULTS = res
    out = np.concatenate([res.results[c]["y"] for c in range(N_CORES)], axis=0)
    return out.reshape(B, C, 32, 32)
